# revision 14
# baseline (speedup 1.0000x reference)
"""Trainium2 kernel for nn_ClementsPSBS (Clements photonic mesh, 1024 layers).

Strategy: the whole network is linear in x (complex transfer matrix), so we
fold all 1024 layers of 2x2 rotations + attenuation into a single complex
matrix T (host-side, cheap), then the HW kernel is out = x @ T^T computed as
two real matmuls distributed over 8 NeuronCores:
  - 4 batch groups (512 rows each) x 2 column groups (real part | imag part)
  - per core: outT[1024, 512] = W[1024(k),1024(n)]^T-style accumulation
    against xT[1024(k), 512(b)] using fp16 matmuls (1 cycle/row).

Schedule (v2): k-outer/n-inner accumulation over all 8 PSUM banks so the
matmul stream starts as soon as the first contraction tile lands; input DMAs
alternate between the two HWDGE rings (Sync / Activation); the PE is
pre-warmed with dummy matmuls during the first DMA's latency so the HAM
p-state ramp overlaps the load; outputs are cast to fp16 (DVE/ACT copies)
and streamed out per-bank on both rings.
"""

import numpy as np

N = 1024          # features
L = 1024          # layers
B = 2048          # batch
NA = N // 2       # pairs per layer
R_GROUPS = 4      # batch groups across cores
C_GROUPS = 2      # column groups (re | im)
BSH = B // R_GROUPS  # 512 batch rows per core
CW = BSH + N      # packed columns per k-row: [xT(512) | W(1024)]

_CACHE = {}
NWARM = 4         # pre-warm dummy matmuls

_NP_DT = np.float16


# ---------------------------------------------------------------------------
# Host-side fold: collapse 1024 layers into one complex transfer matrix T
# such that out = x @ T.T  (T[n, j]: coefficient of input feature j in
# output feature n).
# ---------------------------------------------------------------------------

def _expected_index():
    nA = N // 2
    iA = np.array([[2 * i, 2 * i + 1] for i in range(nA)], dtype=np.int32)
    iB = np.array([[2 * i + 1, 2 * i + 2] for i in range(nA - 1)]
                  + [[~0, ~(N - 1)]], dtype=np.int32)
    layers = [iA if l % 2 == 0 else iB for l in range(L)]
    return np.stack(layers).astype(np.int32)


def _coeffs(params, split, atten, index):
    """Per-layer per-pair 2x2 complex coefficients with attenuation folded in.

    Layer update for pair (p, q):
      u[p]' = at[p]*(cos(a)*e^{i th} * u[p] + i sin(a) * u[q])
      u[q]' = at[q]*(i sin(a)*e^{i th} * u[p] + cos(a) * u[q])
    Rows untouched by a pair still get u *= at.
    """
    theta = params[0].astype(np.float64)          # [L, NA]
    alpha = np.pi / 4 + split.astype(np.float64)  # [L, NA]
    eith = np.exp(1j * theta)
    c = np.cos(alpha)
    s = 1j * np.sin(alpha)
    A = c * eith
    Bc = s + 0j * s
    Cc = s * eith
    D = c + 0j * c
    return A, Bc, Cc, D


def _fold_fast(params, split, atten, index):
    """jax-CPU scan fold for the standard even/odd Clements pattern."""
    import jax
    import jax.numpy as jnp

    A, Bc, Cc, D = _coeffs(params, split, atten, index)
    at = atten.astype(np.complex128)              # [L, N]

    # even layers: pairs (2i, 2i+1), all N rows rotated
    ev = slice(0, L, 2)
    at_p_e = at[ev][:, 0::2]                      # [L/2, NA]
    at_q_e = at[ev][:, 1::2]
    Ae = (A[ev] * at_p_e).astype(np.complex64)
    Be = (Bc[ev] * at_p_e).astype(np.complex64)
    Ce = (Cc[ev] * at_q_e).astype(np.complex64)
    De = (D[ev] * at_q_e).astype(np.complex64)

    # odd layers: pairs (2i+1, 2i+2) for i < NA-1; rows 0 and N-1 only atten
    od = slice(1, L, 2)
    at_p_o = at[od][:, 1:N - 1:2]                 # [L/2, NA-1]
    at_q_o = at[od][:, 2:N:2]
    Ao = (A[od][:, :NA - 1] * at_p_o).astype(np.complex64)
    Bo = (Bc[od][:, :NA - 1] * at_p_o).astype(np.complex64)
    Co = (Cc[od][:, :NA - 1] * at_q_o).astype(np.complex64)
    Do = (D[od][:, :NA - 1] * at_q_o).astype(np.complex64)
    at0 = at[od][:, 0].astype(np.complex64)       # [L/2]
    atN = at[od][:, N - 1].astype(np.complex64)

    cpu = jax.devices('cpu')[0]

    def step(T, co):
        ae, be, ce, de, ao, bo, co_, do, a0, aN = co
        Tr = T.reshape(NA, 2, N)
        p = Tr[:, 0, :]
        q = Tr[:, 1, :]
        np_ = ae[:, None] * p + be[:, None] * q
        nq = ce[:, None] * p + de[:, None] * q
        T = jnp.stack([np_, nq], axis=1).reshape(N, N)
        mid = T[1:N - 1].reshape(NA - 1, 2, N)
        p = mid[:, 0, :]
        q = mid[:, 1, :]
        np_ = ao[:, None] * p + bo[:, None] * q
        nq = co_[:, None] * p + do[:, None] * q
        midn = jnp.stack([np_, nq], axis=1).reshape(N - 2, N)
        T = jnp.concatenate([T[0:1] * a0, midn, T[N - 1:] * aN], axis=0)
        return T, None

    with jax.default_device(cpu):
        T0 = jnp.eye(N, dtype=jnp.complex64)
        coeffs = (Ae, Be, Ce, De, Ao, Bo, Co, Do, at0, atN)
        coeffs = jax.tree.map(jnp.asarray, coeffs)
        fold = jax.jit(lambda T0, co: jax.lax.scan(step, T0, co)[0])
        T = fold(T0, coeffs)
        return np.asarray(T)


def _fold_general(params, split, atten, index):
    """Reference-faithful fold for arbitrary index content (numpy)."""
    A, Bc, Cc, D = _coeffs(params, split, atten, index)
    T = np.eye(N, dtype=np.complex128)
    at = atten.astype(np.complex128)
    for l in range(L):
        idx = index[l]
        valid = (idx >= 0).all(axis=1)
        gi = np.mod(idx, N)
        p = gi[valid, 0]
        q = gi[valid, 1]
        Tp = T[p, :].copy()
        Tq = T[q, :].copy()
        T[p, :] = A[l][valid][:, None] * Tp + Bc[l][valid][:, None] * Tq
        T[q, :] = Cc[l][valid][:, None] * Tp + D[l][valid][:, None] * Tq
        T *= at[l][:, None]
    return T.astype(np.complex64)


def _fold(params, split, atten, index):
    if np.array_equal(index, _expected_index()):
        try:
            return _fold_fast(params, split, atten, index)
        except Exception:
            pass
    return _fold_general(params, split, atten, index)


# ---------------------------------------------------------------------------
# Device kernel: outT = accumulate_k W[k,:].T @ xT[k,:] per core
# ---------------------------------------------------------------------------

def _build_nc():
    import concourse.bass as bass
    import concourse.bacc as bacc
    import concourse.mybir as mybir
    import concourse.tile as tile
    from contextlib import ExitStack

    f32 = mybir.dt.float32
    f16 = mybir.dt.float16

    nc = bacc.Bacc("TRN2", target_bir_lowering=False, debug=False,
                   num_devices=8, enable_partition_id=False)
    # packed input: per contraction row j, [xT_shard(512) | W(1024)] columns
    XW = nc.dram_tensor("XW", [N, CW], f16, kind="ExternalInput").ap()
    # output packed as bank-pairs: row block j holds features
    # [256j..256j+127] in cols 0:512 and [256j+128..256j+255] in 512:1024,
    # so a two-bank SBUF slice maps to one shape-preserving DMA.
    OUT = nc.dram_tensor("OUT", [N // 2, 2 * BSH], f16,
                         kind="ExternalOutput").ap()

    KT = N // 128   # 8 contraction tiles
    NT = N // 128   # 8 output column tiles

    with tile.TileContext(nc) as tc, ExitStack() as ctx:
        xwpool = ctx.enter_context(tc.tile_pool(name="xwp", bufs=1))
        opool = ctx.enter_context(tc.tile_pool(name="op", bufs=1))
        wpool = ctx.enter_context(tc.tile_pool(name="wp", bufs=1))
        ppool = ctx.enter_context(tc.tile_pool(name="pp", bufs=1, space="PSUM"))

        # PE pre-warm: dummy matmuls from a memset scratch tile run during
        # the first input DMA's latency so the HAM p-state ramp (~3.4us at
        # 1.2 GHz) overlaps the load instead of the real matmul stream.
        warm = wpool.tile([128, BSH], f16, name="warm")
        nc.gpsimd.memset(warm[:], 0.0)

        # one PSUM tensor spanning all 8 banks; each 512-col slice is one
        # bank (matmul outputs must stay within a bank)
        ps = ppool.tile([128, NT * BSH], f32, name="ps")

        for i in range(NWARM):
            nc.tensor.matmul(ps[:, 0:BSH], warm[:, 0:128], warm[:, 0:BSH],
                             start=True, stop=True)

        # input DMAs all on the sync (SP) HWDGE ring so tiles arrive in
        # strict k order at full HBM bandwidth (two rings would split the
        # bandwidth and make pairs of tiles land together, starving the
        # matmul stream early). The k=0 tile is split so the first matmul
        # (needs x + W column block 0) can start before the rest lands.
        # The first tiles are split in half so the DMA-completion latency
        # (~0.9us semaphore propagation) applies to half-sized transfers
        # and the matmul stream isn't starved while it's still cold.
        HC = CW // 2   # 768 columns: [x(512) | W cols 0:256] / [W cols 256:1024]
        xwts = []
        for k in range(KT):
            xwt = xwpool.tile([128, CW], f16, tag=f"xw{k}", name=f"xw{k}")
            rs = 128 * k
            if k < 2:
                nc.sync.dma_start(out=xwt[:, 0:HC], in_=XW[rs:rs + 128, 0:HC])
                nc.sync.dma_start(out=xwt[:, HC:CW], in_=XW[rs:rs + 128, HC:CW])
            else:
                nc.sync.dma_start(out=xwt[:], in_=XW[rs:rs + 128, :])
            xwts.append(xwt)

        # k-outer / n-inner for k=0..5 so the matmul stream consumes each
        # contraction tile as it lands; each bank's last two tiles (k=6,7)
        # are emitted bank-major at the end so banks CLOSE staggered
        # (~0.43us apart) and the psum-evacuation pipeline overlaps the
        # tail of the matmul stream instead of all chasing the last tile.
        def emit_mm(n, k):
            nc.tensor.matmul(
                ps[:, BSH * n:BSH * (n + 1)],
                xwts[k][:, BSH + 128 * n:BSH + 128 * (n + 1)],
                xwts[k][:, 0:BSH],
                start=(k == 0),
                stop=(k == KT - 1),
            )

        osb = opool.tile([128, NT * BSH], f16, name="osb")

        def emit_cast(n):
            src = ps[:, n * BSH:(n + 1) * BSH]
            dst = osb[:, n * BSH:(n + 1) * BSH]
            if n % 2 == 0:
                nc.vector.tensor_copy(dst, src)
            else:
                nc.scalar.copy(dst, src)

        def emit_out(n0, n1, eng=None):
            # bank-pair j sits at OUT rows [128j:128j+128]; a single bank n
            # maps to the (n%2) column half of its pair's row block.
            j = n0 // 2
            c0 = (n0 % 2) * BSH
            c1 = c0 + (n1 - n0) * BSH
            (eng or nc.sync).dma_start(out=OUT[128 * j:128 * (j + 1), c0:c1],
                                       in_=osb[:, n0 * BSH:n1 * BSH])

        for k in range(KT - 2):
            for n in range(NT):
                emit_mm(n, k)
        # banks close staggered; casts alternate DVE/ACT; output DMAs all
        # on the sync ring (idle after the input issues): pairs for the
        # early banks to cut issue count, singles for the last two so the
        # final transfer is small.
        for n in range(NT):
            emit_mm(n, KT - 2)
            emit_mm(n, KT - 1)
            emit_cast(n)
            if n in (1, 3, 5):
                emit_out(n - 1, n + 1)
            elif n == 6:
                emit_out(6, 7)
            elif n == 7:
                # last bank issues from the ACT ring right after its own
                # cast instead of queueing behind sync's earlier issues
                emit_out(7, 8, eng=nc.scalar)

    nc.compile()
    return nc


def _get_nc():
    if "nc" not in _CACHE:
        _CACHE["nc"] = _build_nc()
    return _CACHE["nc"]


def kernel(x, params, split, atten, index):
    from concourse.bass_utils import run_bass_kernel_spmd

    x = np.asarray(x, dtype=np.float32)
    T = _fold(np.asarray(params), np.asarray(split), np.asarray(atten),
              np.asarray(index))

    # W[j, n] = T[n, j] so that out[b, n] = sum_j xT[j, b] * W[j, n]
    Wre = T.real.T.astype(_NP_DT)
    Wim = T.imag.T.astype(_NP_DT)
    xTfull = x.T.astype(_NP_DT)           # [N, B]

    nc = _get_nc()
    in_maps = []
    for core in range(8):
        bg, cg = divmod(core, C_GROUPS)
        xw = np.empty((N, CW), dtype=_NP_DT)
        xw[:, :BSH] = xTfull[:, bg * BSH:(bg + 1) * BSH]
        xw[:, BSH:] = Wre if cg == 0 else Wim
        in_maps.append({"XW": xw})
    res = run_bass_kernel_spmd(nc, in_maps, list(range(8)))

    out = np.empty((B, N), dtype=np.complex64)
    for core in range(8):
        bg, cg = divmod(core, C_GROUPS)
        o2 = res.results[core]["OUT"]                    # [N//2, 2*BSH] fp16
        o = (o2.reshape(4, 128, 2, BSH).transpose(0, 2, 1, 3)
               .reshape(N, BSH).astype(np.float32))      # [N, BSH]
        if cg == 0:
            out.real[bg * BSH:(bg + 1) * BSH, :] = o.T
        else:
            out.imag[bg * BSH:(bg + 1) * BSH, :] = o.T
    return out


# revision 17
# speedup vs baseline: 1.0517x; 1.0517x over previous
"""Trainium2 kernel for nn_ClementsPSBS (Clements photonic mesh, 1024 layers).

Strategy: the whole network is linear in x (complex transfer matrix), so we
fold all 1024 layers of 2x2 rotations + attenuation into a single complex
matrix T (host-side, cheap), then the HW kernel is out = x @ T^T computed as
two real matmuls distributed over 8 NeuronCores:
  - 4 batch groups (512 rows each) x 2 column groups (real part | imag part)
  - per core: outT[1024, 512] = W[1024(k),1024(n)]^T-style accumulation
    against xT[1024(k), 512(b)] using fp16 matmuls (1 cycle/row).

Schedule (v2): k-outer/n-inner accumulation over all 8 PSUM banks so the
matmul stream starts as soon as the first contraction tile lands; input DMAs
alternate between the two HWDGE rings (Sync / Activation); the PE is
pre-warmed with dummy matmuls during the first DMA's latency so the HAM
p-state ramp overlaps the load; outputs are cast to fp16 (DVE/ACT copies)
and streamed out per-bank on both rings.
"""

import numpy as np

N = 1024          # features
L = 1024          # layers
B = 2048          # batch
NA = N // 2       # pairs per layer
R_GROUPS = 4      # batch groups across cores
C_GROUPS = 2      # column groups (re | im)
BSH = B // R_GROUPS  # 512 batch rows per core
CW = BSH + N      # packed columns per k-row: [xT(512) | W(1024)]

_CACHE = {}
NWARM = 10        # pre-warm dummy matmuls (free dim 256 each)

_NP_DT = np.float16


# ---------------------------------------------------------------------------
# Host-side fold: collapse 1024 layers into one complex transfer matrix T
# such that out = x @ T.T  (T[n, j]: coefficient of input feature j in
# output feature n).
# ---------------------------------------------------------------------------

def _expected_index():
    nA = N // 2
    iA = np.array([[2 * i, 2 * i + 1] for i in range(nA)], dtype=np.int32)
    iB = np.array([[2 * i + 1, 2 * i + 2] for i in range(nA - 1)]
                  + [[~0, ~(N - 1)]], dtype=np.int32)
    layers = [iA if l % 2 == 0 else iB for l in range(L)]
    return np.stack(layers).astype(np.int32)


def _coeffs(params, split, atten, index):
    """Per-layer per-pair 2x2 complex coefficients with attenuation folded in.

    Layer update for pair (p, q):
      u[p]' = at[p]*(cos(a)*e^{i th} * u[p] + i sin(a) * u[q])
      u[q]' = at[q]*(i sin(a)*e^{i th} * u[p] + cos(a) * u[q])
    Rows untouched by a pair still get u *= at.
    """
    theta = params[0].astype(np.float64)          # [L, NA]
    alpha = np.pi / 4 + split.astype(np.float64)  # [L, NA]
    eith = np.exp(1j * theta)
    c = np.cos(alpha)
    s = 1j * np.sin(alpha)
    A = c * eith
    Bc = s + 0j * s
    Cc = s * eith
    D = c + 0j * c
    return A, Bc, Cc, D


def _fold_fast(params, split, atten, index):
    """jax-CPU scan fold for the standard even/odd Clements pattern."""
    import jax
    import jax.numpy as jnp

    A, Bc, Cc, D = _coeffs(params, split, atten, index)
    at = atten.astype(np.complex128)              # [L, N]

    # even layers: pairs (2i, 2i+1), all N rows rotated
    ev = slice(0, L, 2)
    at_p_e = at[ev][:, 0::2]                      # [L/2, NA]
    at_q_e = at[ev][:, 1::2]
    Ae = (A[ev] * at_p_e).astype(np.complex64)
    Be = (Bc[ev] * at_p_e).astype(np.complex64)
    Ce = (Cc[ev] * at_q_e).astype(np.complex64)
    De = (D[ev] * at_q_e).astype(np.complex64)

    # odd layers: pairs (2i+1, 2i+2) for i < NA-1; rows 0 and N-1 only atten
    od = slice(1, L, 2)
    at_p_o = at[od][:, 1:N - 1:2]                 # [L/2, NA-1]
    at_q_o = at[od][:, 2:N:2]
    Ao = (A[od][:, :NA - 1] * at_p_o).astype(np.complex64)
    Bo = (Bc[od][:, :NA - 1] * at_p_o).astype(np.complex64)
    Co = (Cc[od][:, :NA - 1] * at_q_o).astype(np.complex64)
    Do = (D[od][:, :NA - 1] * at_q_o).astype(np.complex64)
    at0 = at[od][:, 0].astype(np.complex64)       # [L/2]
    atN = at[od][:, N - 1].astype(np.complex64)

    cpu = jax.devices('cpu')[0]

    def step(T, co):
        ae, be, ce, de, ao, bo, co_, do, a0, aN = co
        Tr = T.reshape(NA, 2, N)
        p = Tr[:, 0, :]
        q = Tr[:, 1, :]
        np_ = ae[:, None] * p + be[:, None] * q
        nq = ce[:, None] * p + de[:, None] * q
        T = jnp.stack([np_, nq], axis=1).reshape(N, N)
        mid = T[1:N - 1].reshape(NA - 1, 2, N)
        p = mid[:, 0, :]
        q = mid[:, 1, :]
        np_ = ao[:, None] * p + bo[:, None] * q
        nq = co_[:, None] * p + do[:, None] * q
        midn = jnp.stack([np_, nq], axis=1).reshape(N - 2, N)
        T = jnp.concatenate([T[0:1] * a0, midn, T[N - 1:] * aN], axis=0)
        return T, None

    with jax.default_device(cpu):
        T0 = jnp.eye(N, dtype=jnp.complex64)
        coeffs = (Ae, Be, Ce, De, Ao, Bo, Co, Do, at0, atN)
        coeffs = jax.tree.map(jnp.asarray, coeffs)
        fold = jax.jit(lambda T0, co: jax.lax.scan(step, T0, co)[0])
        T = fold(T0, coeffs)
        return np.asarray(T)


def _fold_general(params, split, atten, index):
    """Reference-faithful fold for arbitrary index content (numpy)."""
    A, Bc, Cc, D = _coeffs(params, split, atten, index)
    T = np.eye(N, dtype=np.complex128)
    at = atten.astype(np.complex128)
    for l in range(L):
        idx = index[l]
        valid = (idx >= 0).all(axis=1)
        gi = np.mod(idx, N)
        p = gi[valid, 0]
        q = gi[valid, 1]
        Tp = T[p, :].copy()
        Tq = T[q, :].copy()
        T[p, :] = A[l][valid][:, None] * Tp + Bc[l][valid][:, None] * Tq
        T[q, :] = Cc[l][valid][:, None] * Tp + D[l][valid][:, None] * Tq
        T *= at[l][:, None]
    return T.astype(np.complex64)


def _fold(params, split, atten, index):
    if np.array_equal(index, _expected_index()):
        try:
            return _fold_fast(params, split, atten, index)
        except Exception:
            pass
    return _fold_general(params, split, atten, index)


# ---------------------------------------------------------------------------
# Device kernel: outT = accumulate_k W[k,:].T @ xT[k,:] per core
# ---------------------------------------------------------------------------

def _build_nc():
    import concourse.bass as bass
    import concourse.bacc as bacc
    import concourse.mybir as mybir
    import concourse.tile as tile
    from contextlib import ExitStack

    f32 = mybir.dt.float32
    f16 = mybir.dt.float16

    nc = bacc.Bacc("TRN2", target_bir_lowering=False, debug=False,
                   num_devices=8)
    # packed input: per contraction row j, [xT_shard(512) | W(1024)] columns
    XW = nc.dram_tensor("XW", [N, CW], f16, kind="ExternalInput").ap()
    # output packed as bank-pairs: row block j holds features
    # [256j..256j+127] in cols 0:512 and [256j+128..256j+255] in 512:1024,
    # so a two-bank SBUF slice maps to one shape-preserving DMA.
    OUT = nc.dram_tensor("OUT", [N // 2, 2 * BSH], f16,
                         kind="ExternalOutput").ap()

    KT = N // 128   # 8 contraction tiles
    NT = N // 128   # 8 output column tiles

    with tile.TileContext(nc) as tc, ExitStack() as ctx:
        xwpool = ctx.enter_context(tc.tile_pool(name="xwp", bufs=1))
        opool = ctx.enter_context(tc.tile_pool(name="op", bufs=1))
        wpool = ctx.enter_context(tc.tile_pool(name="wp", bufs=1))
        ppool = ctx.enter_context(tc.tile_pool(name="pp", bufs=1, space="PSUM"))

        # PE pre-warm: dummy matmuls from a memset scratch tile run during
        # the first input DMA's latency so the HAM p-state ramp (~3.4us at
        # 1.2 GHz) overlaps the load instead of the real matmul stream.
        warm = wpool.tile([128, BSH], f16, name="warm")
        nc.gpsimd.memset(warm[:], 0.0)

        # one PSUM tensor spanning all 8 banks; each 512-col slice is one
        # bank (matmul outputs must stay within a bank)
        ps = ppool.tile([128, NT * BSH], f32, name="ps")

        for i in range(NWARM):
            nc.tensor.matmul(ps[:, 0:256], warm[:, 0:128], warm[:, 0:256],
                             start=True, stop=True)

        # input DMAs all on the sync (SP) HWDGE ring so tiles arrive in
        # strict k order at full HBM bandwidth (two rings would split the
        # bandwidth and make pairs of tiles land together, starving the
        # matmul stream early). The k=0 tile is split so the first matmul
        # (needs x + W column block 0) can start before the rest lands.
        # The first tiles are split in half so the DMA-completion latency
        # (~0.9us semaphore propagation) applies to half-sized transfers
        # and the matmul stream isn't starved while it's still cold.
        HC = CW // 2   # 768 columns: [x(512) | W cols 0:256] / [W cols 256:1024]
        xwts = []
        for k in range(KT):
            xwt = xwpool.tile([128, CW], f16, tag=f"xw{k}", name=f"xw{k}")
            rs = 128 * k
            if k < 2:
                nc.sync.dma_start(out=xwt[:, 0:HC], in_=XW[rs:rs + 128, 0:HC])
                nc.sync.dma_start(out=xwt[:, HC:CW], in_=XW[rs:rs + 128, HC:CW])
            else:
                nc.sync.dma_start(out=xwt[:], in_=XW[rs:rs + 128, :])
            xwts.append(xwt)

        # k-outer / n-inner for k=0..5 so the matmul stream consumes each
        # contraction tile as it lands; each bank's last two tiles (k=6,7)
        # are emitted bank-major at the end so banks CLOSE staggered
        # (~0.43us apart) and the psum-evacuation pipeline overlaps the
        # tail of the matmul stream instead of all chasing the last tile.
        def emit_mm(n, k):
            nc.tensor.matmul(
                ps[:, BSH * n:BSH * (n + 1)],
                xwts[k][:, BSH + 128 * n:BSH + 128 * (n + 1)],
                xwts[k][:, 0:BSH],
                start=(k == 0),
                stop=(k == KT - 1),
            )

        osb = opool.tile([128, NT * BSH], f16, name="osb")

        def emit_cast(n):
            src = ps[:, n * BSH:(n + 1) * BSH]
            dst = osb[:, n * BSH:(n + 1) * BSH]
            if n % 2 == 0:
                nc.vector.tensor_copy(dst, src)
            else:
                nc.scalar.copy(dst, src)

        def emit_out(n0, n1, eng=None):
            # bank-pair j sits at OUT rows [128j:128j+128]; a single bank n
            # maps to the (n%2) column half of its pair's row block.
            j = n0 // 2
            c0 = (n0 % 2) * BSH
            c1 = c0 + (n1 - n0) * BSH
            (eng or nc.sync).dma_start(out=OUT[128 * j:128 * (j + 1), c0:c1],
                                       in_=osb[:, n0 * BSH:n1 * BSH])

        for k in range(KT - 2):
            for n in range(NT):
                emit_mm(n, k)
        # banks close staggered; casts alternate DVE/ACT; output DMAs all
        # on the sync ring (idle after the input issues): pairs for the
        # early banks to cut issue count, singles for the last two so the
        # final transfer is small.
        for n in range(NT):
            emit_mm(n, KT - 2)
            emit_mm(n, KT - 1)
            emit_cast(n)
            if n in (1, 3, 5):
                emit_out(n - 1, n + 1)
            elif n == 6:
                emit_out(6, 7)
            elif n == 7:
                # last bank issues from the ACT ring right after its own
                # cast instead of queueing behind sync's earlier issues
                emit_out(7, 8, eng=nc.scalar)

    nc.compile()
    return nc


def _get_nc():
    if "nc" not in _CACHE:
        _CACHE["nc"] = _build_nc()
    return _CACHE["nc"]


def kernel(x, params, split, atten, index):
    from concourse.bass_utils import run_bass_kernel_spmd

    x = np.asarray(x, dtype=np.float32)
    T = _fold(np.asarray(params), np.asarray(split), np.asarray(atten),
              np.asarray(index))

    # W[j, n] = T[n, j] so that out[b, n] = sum_j xT[j, b] * W[j, n]
    Wre = T.real.T.astype(_NP_DT)
    Wim = T.imag.T.astype(_NP_DT)
    xTfull = x.T.astype(_NP_DT)           # [N, B]

    nc = _get_nc()
    in_maps = []
    for core in range(8):
        bg, cg = divmod(core, C_GROUPS)
        xw = np.empty((N, CW), dtype=_NP_DT)
        xw[:, :BSH] = xTfull[:, bg * BSH:(bg + 1) * BSH]
        xw[:, BSH:] = Wre if cg == 0 else Wim
        in_maps.append({"XW": xw})
    res = run_bass_kernel_spmd(nc, in_maps, list(range(8)))

    out = np.empty((B, N), dtype=np.complex64)
    for core in range(8):
        bg, cg = divmod(core, C_GROUPS)
        o2 = res.results[core]["OUT"]                    # [N//2, 2*BSH] fp16
        o = (o2.reshape(4, 128, 2, BSH).transpose(0, 2, 1, 3)
               .reshape(N, BSH).astype(np.float32))      # [N, BSH]
        if cg == 0:
            out.real[bg * BSH:(bg + 1) * BSH, :] = o.T
        else:
            out.imag[bg * BSH:(bg + 1) * BSH, :] = o.T
    return out


# revision 20
# speedup vs baseline: 1.0545x; 1.0026x over previous
"""Trainium2 kernel for nn_ClementsPSBS (Clements photonic mesh, 1024 layers).

Strategy: the whole network is linear in x (complex transfer matrix), so we
fold all 1024 layers of 2x2 rotations + attenuation into a single complex
matrix T (host-side, cheap), then the HW kernel is out = x @ T^T computed as
two real matmuls distributed over 8 NeuronCores:
  - 4 batch groups (512 rows each) x 2 column groups (real part | imag part)
  - per core: outT[1024, 512] = W[1024(k),1024(n)]^T-style accumulation
    against xT[1024(k), 512(b)] using fp16 matmuls (1 cycle/row).

Schedule (v2): k-outer/n-inner accumulation over all 8 PSUM banks so the
matmul stream starts as soon as the first contraction tile lands; input DMAs
alternate between the two HWDGE rings (Sync / Activation); the PE is
pre-warmed with dummy matmuls during the first DMA's latency so the HAM
p-state ramp overlaps the load; outputs are cast to fp16 (DVE/ACT copies)
and streamed out per-bank on both rings.
"""

import numpy as np

N = 1024          # features
L = 1024          # layers
B = 2048          # batch
NA = N // 2       # pairs per layer
R_GROUPS = 4      # batch groups across cores
C_GROUPS = 2      # column groups (re | im)
BSH = B // R_GROUPS  # 512 batch rows per core
CW = BSH + N      # packed columns per k-row: [xT(512) | W(1024)]

_CACHE = {}
NWARM = 10        # pre-warm dummy matmuls (free dim 256 each)

_NP_DT = np.float16


# ---------------------------------------------------------------------------
# Host-side fold: collapse 1024 layers into one complex transfer matrix T
# such that out = x @ T.T  (T[n, j]: coefficient of input feature j in
# output feature n).
# ---------------------------------------------------------------------------

def _expected_index():
    nA = N // 2
    iA = np.array([[2 * i, 2 * i + 1] for i in range(nA)], dtype=np.int32)
    iB = np.array([[2 * i + 1, 2 * i + 2] for i in range(nA - 1)]
                  + [[~0, ~(N - 1)]], dtype=np.int32)
    layers = [iA if l % 2 == 0 else iB for l in range(L)]
    return np.stack(layers).astype(np.int32)


def _coeffs(params, split, atten, index):
    """Per-layer per-pair 2x2 complex coefficients with attenuation folded in.

    Layer update for pair (p, q):
      u[p]' = at[p]*(cos(a)*e^{i th} * u[p] + i sin(a) * u[q])
      u[q]' = at[q]*(i sin(a)*e^{i th} * u[p] + cos(a) * u[q])
    Rows untouched by a pair still get u *= at.
    """
    theta = params[0].astype(np.float64)          # [L, NA]
    alpha = np.pi / 4 + split.astype(np.float64)  # [L, NA]
    eith = np.exp(1j * theta)
    c = np.cos(alpha)
    s = 1j * np.sin(alpha)
    A = c * eith
    Bc = s + 0j * s
    Cc = s * eith
    D = c + 0j * c
    return A, Bc, Cc, D


def _fold_fast(params, split, atten, index):
    """jax-CPU scan fold for the standard even/odd Clements pattern."""
    import jax
    import jax.numpy as jnp

    A, Bc, Cc, D = _coeffs(params, split, atten, index)
    at = atten.astype(np.complex128)              # [L, N]

    # even layers: pairs (2i, 2i+1), all N rows rotated
    ev = slice(0, L, 2)
    at_p_e = at[ev][:, 0::2]                      # [L/2, NA]
    at_q_e = at[ev][:, 1::2]
    Ae = (A[ev] * at_p_e).astype(np.complex64)
    Be = (Bc[ev] * at_p_e).astype(np.complex64)
    Ce = (Cc[ev] * at_q_e).astype(np.complex64)
    De = (D[ev] * at_q_e).astype(np.complex64)

    # odd layers: pairs (2i+1, 2i+2) for i < NA-1; rows 0 and N-1 only atten
    od = slice(1, L, 2)
    at_p_o = at[od][:, 1:N - 1:2]                 # [L/2, NA-1]
    at_q_o = at[od][:, 2:N:2]
    Ao = (A[od][:, :NA - 1] * at_p_o).astype(np.complex64)
    Bo = (Bc[od][:, :NA - 1] * at_p_o).astype(np.complex64)
    Co = (Cc[od][:, :NA - 1] * at_q_o).astype(np.complex64)
    Do = (D[od][:, :NA - 1] * at_q_o).astype(np.complex64)
    at0 = at[od][:, 0].astype(np.complex64)       # [L/2]
    atN = at[od][:, N - 1].astype(np.complex64)

    cpu = jax.devices('cpu')[0]

    def step(T, co):
        ae, be, ce, de, ao, bo, co_, do, a0, aN = co
        Tr = T.reshape(NA, 2, N)
        p = Tr[:, 0, :]
        q = Tr[:, 1, :]
        np_ = ae[:, None] * p + be[:, None] * q
        nq = ce[:, None] * p + de[:, None] * q
        T = jnp.stack([np_, nq], axis=1).reshape(N, N)
        mid = T[1:N - 1].reshape(NA - 1, 2, N)
        p = mid[:, 0, :]
        q = mid[:, 1, :]
        np_ = ao[:, None] * p + bo[:, None] * q
        nq = co_[:, None] * p + do[:, None] * q
        midn = jnp.stack([np_, nq], axis=1).reshape(N - 2, N)
        T = jnp.concatenate([T[0:1] * a0, midn, T[N - 1:] * aN], axis=0)
        return T, None

    with jax.default_device(cpu):
        T0 = jnp.eye(N, dtype=jnp.complex64)
        coeffs = (Ae, Be, Ce, De, Ao, Bo, Co, Do, at0, atN)
        coeffs = jax.tree.map(jnp.asarray, coeffs)
        fold = jax.jit(lambda T0, co: jax.lax.scan(step, T0, co)[0])
        T = fold(T0, coeffs)
        return np.asarray(T)


def _fold_general(params, split, atten, index):
    """Reference-faithful fold for arbitrary index content (numpy)."""
    A, Bc, Cc, D = _coeffs(params, split, atten, index)
    T = np.eye(N, dtype=np.complex128)
    at = atten.astype(np.complex128)
    for l in range(L):
        idx = index[l]
        valid = (idx >= 0).all(axis=1)
        gi = np.mod(idx, N)
        p = gi[valid, 0]
        q = gi[valid, 1]
        Tp = T[p, :].copy()
        Tq = T[q, :].copy()
        T[p, :] = A[l][valid][:, None] * Tp + Bc[l][valid][:, None] * Tq
        T[q, :] = Cc[l][valid][:, None] * Tp + D[l][valid][:, None] * Tq
        T *= at[l][:, None]
    return T.astype(np.complex64)


def _fold(params, split, atten, index):
    if np.array_equal(index, _expected_index()):
        try:
            return _fold_fast(params, split, atten, index)
        except Exception:
            pass
    return _fold_general(params, split, atten, index)


# ---------------------------------------------------------------------------
# Device kernel: outT = accumulate_k W[k,:].T @ xT[k,:] per core
# ---------------------------------------------------------------------------

def _build_nc():
    import concourse.bass as bass
    import concourse.bacc as bacc
    import concourse.mybir as mybir
    import concourse.tile as tile
    from contextlib import ExitStack

    f32 = mybir.dt.float32
    f16 = mybir.dt.float16

    nc = bacc.Bacc("TRN2", target_bir_lowering=False, debug=False,
                   num_devices=8)
    # packed input: per contraction row j, [xT_shard(512) | W(1024)] columns
    XW = nc.dram_tensor("XW", [N, CW], f16, kind="ExternalInput").ap()
    # output packed as bank-pairs: row block j holds features
    # [256j..256j+127] in cols 0:512 and [256j+128..256j+255] in 512:1024,
    # so a two-bank SBUF slice maps to one shape-preserving DMA.
    OUT = nc.dram_tensor("OUT", [N // 2, 2 * BSH], f16,
                         kind="ExternalOutput").ap()

    KT = N // 128   # 8 contraction tiles
    NT = N // 128   # 8 output column tiles

    with tile.TileContext(nc) as tc, ExitStack() as ctx:
        xwpool = ctx.enter_context(tc.tile_pool(name="xwp", bufs=1))
        opool = ctx.enter_context(tc.tile_pool(name="op", bufs=1))
        wpool = ctx.enter_context(tc.tile_pool(name="wp", bufs=1))
        ppool = ctx.enter_context(tc.tile_pool(name="pp", bufs=1, space="PSUM"))

        # PE pre-warm: dummy matmuls from a memset scratch tile run during
        # the first input DMA's latency so the HAM p-state ramp (~3.4us at
        # 1.2 GHz) overlaps the load instead of the real matmul stream.
        warm = wpool.tile([128, BSH], f16, name="warm")
        nc.gpsimd.memset(warm[:], 0.0)

        # one PSUM tensor spanning all 8 banks; each 512-col slice is one
        # bank (matmul outputs must stay within a bank)
        ps = ppool.tile([128, NT * BSH], f32, name="ps")

        for i in range(NWARM):
            nc.tensor.matmul(ps[:, 0:256], warm[:, 0:128], warm[:, 0:256],
                             start=True, stop=True)

        # input DMAs all on the sync (SP) HWDGE ring so tiles arrive in
        # strict k order at full HBM bandwidth (two rings would split the
        # bandwidth and make pairs of tiles land together, starving the
        # matmul stream early). The k=0 tile is split so the first matmul
        # (needs x + W column block 0) can start before the rest lands.
        # One SBUF tile holds all 8 contraction tiles side by side (fewer
        # tiles = fewer drain-time semaphore resets); DMA k writes columns
        # [CW*k, CW*(k+1)). The first tiles are split in half so the
        # DMA-completion latency (~0.9us semaphore propagation) applies to
        # half-sized transfers and the cold matmul stream isn't starved.
        HC = CW // 2   # 768 columns: [x(512) | W cols 0:256] / [W cols 256:1024]
        xw = xwpool.tile([128, KT * CW], f16, name="xw")
        for k in range(KT):
            rs = 128 * k
            cb = CW * k
            if k < 2:
                nc.sync.dma_start(out=xw[:, cb:cb + HC],
                                  in_=XW[rs:rs + 128, 0:HC])
                nc.sync.dma_start(out=xw[:, cb + HC:cb + CW],
                                  in_=XW[rs:rs + 128, HC:CW])
            else:
                nc.sync.dma_start(out=xw[:, cb:cb + CW],
                                  in_=XW[rs:rs + 128, :])

        def emit_mm(n, k):
            nc.tensor.matmul(
                ps[:, BSH * n:BSH * (n + 1)],
                xw[:, CW * k + BSH + 128 * n:CW * k + BSH + 128 * (n + 1)],
                xw[:, CW * k:CW * k + BSH],
                start=(k == 0),
                stop=(k == KT - 1),
            )

        osb = opool.tile([128, NT * BSH], f16, name="osb")

        def emit_cast(n):
            src = ps[:, n * BSH:(n + 1) * BSH]
            dst = osb[:, n * BSH:(n + 1) * BSH]
            if n % 2 == 0:
                nc.vector.tensor_copy(dst, src)
            else:
                nc.scalar.copy(dst, src)

        def emit_out(n0, n1, eng=None):
            # bank-pair j sits at OUT rows [128j:128j+128]; a single bank n
            # maps to the (n%2) column half of its pair's row block.
            j = n0 // 2
            c0 = (n0 % 2) * BSH
            c1 = c0 + (n1 - n0) * BSH
            (eng or nc.sync).dma_start(out=OUT[128 * j:128 * (j + 1), c0:c1],
                                       in_=osb[:, n0 * BSH:n1 * BSH])

        # k-outer / n-inner for k=0..5 so the matmul stream consumes each
        # contraction tile as it lands; each bank's last two tiles (k=6,7)
        # are emitted bank-major at the end so banks CLOSE staggered
        # (~0.43us apart) and the psum-evacuation pipeline overlaps the
        # tail of the matmul stream instead of all chasing the last tile.
        for k in range(KT - 2):
            for n in range(NT):
                emit_mm(n, k)
        for n in range(NT):
            emit_mm(n, KT - 2)
            emit_mm(n, KT - 1)
            emit_cast(n)
            if n in (1, 3, 5):
                emit_out(n - 1, n + 1)
            elif n == 6:
                emit_out(6, 7)
            elif n == 7:
                # last bank issues from the ACT ring right after its own
                # cast instead of queueing behind sync's earlier issues
                emit_out(7, 8, eng=nc.scalar)

    nc.compile()
    return nc


def _get_nc():
    if "nc" not in _CACHE:
        _CACHE["nc"] = _build_nc()
    return _CACHE["nc"]


def kernel(x, params, split, atten, index):
    from concourse.bass_utils import run_bass_kernel_spmd

    x = np.asarray(x, dtype=np.float32)
    T = _fold(np.asarray(params), np.asarray(split), np.asarray(atten),
              np.asarray(index))

    # W[j, n] = T[n, j] so that out[b, n] = sum_j xT[j, b] * W[j, n]
    Wre = T.real.T.astype(_NP_DT)
    Wim = T.imag.T.astype(_NP_DT)
    xTfull = x.T.astype(_NP_DT)           # [N, B]

    nc = _get_nc()
    in_maps = []
    for core in range(8):
        bg, cg = divmod(core, C_GROUPS)
        xw = np.empty((N, CW), dtype=_NP_DT)
        xw[:, :BSH] = xTfull[:, bg * BSH:(bg + 1) * BSH]
        xw[:, BSH:] = Wre if cg == 0 else Wim
        in_maps.append({"XW": xw})
    res = run_bass_kernel_spmd(nc, in_maps, list(range(8)))

    out = np.empty((B, N), dtype=np.complex64)
    for core in range(8):
        bg, cg = divmod(core, C_GROUPS)
        o2 = res.results[core]["OUT"]                    # [N//2, 2*BSH] fp16
        o = (o2.reshape(4, 128, 2, BSH).transpose(0, 2, 1, 3)
               .reshape(N, BSH).astype(np.float32))      # [N, BSH]
        if cg == 0:
            out.real[bg * BSH:(bg + 1) * BSH, :] = o.T
        else:
            out.imag[bg * BSH:(bg + 1) * BSH, :] = o.T
    return out
